# revision 55
# baseline (speedup 1.0000x reference)
"""Multi-head attention block (qkv proj -> softmax attention -> out proj)
for B=2, N=2048, C=1024, H=16 heads of d=64, distributed over 8 NeuronCores.

Sharding: core c = (b, g) with b = c // 4 (batch), g = c % 4 (head group of
4 heads). Each core computes q/k/v for its 4 heads, full softmax attention,
and a partial output projection (its 256 input channels of w_proj). The
host sums the 4 per-batch partials and adds b_proj.

Device layout notes (per core):
  - xT [1024, 2048] = x[b].T so the contraction dim (C) lands on SBUF
    partitions for both qkv orientations.
  - q/k are produced transposed ([head_dim, tokens]); consecutive heads sit
    at partition offsets 0 / 64 so the two K=64 score matmuls of a head
    pair occupy disjoint PE row groups and run concurrently (row tiling).
  - v is produced in [tokens, head_dim] layout with an extra all-ones
    column per head; the PV matmul then yields both the unnormalized
    attention output and the softmax denominator Z in one pass.
  - softmax has no max-subtraction: scores are ~N(0,1) (|S*scale| < ~8),
    safely inside fp32 exp range.
"""

import sys
import types

import numpy as np
import ml_dtypes

B = 2
N = 2048
C = 1024
H = 16
D = 64
HL = H // 4          # heads per core = 4
SCALE = D ** -0.5
N_CORES = 8
KT = C // 128        # 8 contraction tiles
MT = N // 128        # 16 token tiles
BF = ml_dtypes.bfloat16

_cache = {}


def _install_ntff_hook():
    """Register the axon NTFF profiling hook that this image's antenv lacks
    (profiling degrades gracefully without it; needed for exec_time_ns)."""
    try:
        import antenv.axon_hooks  # noqa: F401
        return
    except ImportError:
        pass
    try:
        import antenv
        from trn_agent_boot.trn_boot import _ntff_profile_via_ctypes
    except ImportError:
        return
    mod = types.ModuleType("antenv.axon_hooks")
    _hook = [None]
    mod.set_axon_ntff_profile_hook = lambda h: _hook.__setitem__(0, h)
    mod.get_axon_ntff_profile_hook = lambda: _hook[0]
    sys.modules["antenv.axon_hooks"] = mod
    antenv.axon_hooks = mod
    try:
        mod.set_axon_ntff_profile_hook(
            _ntff_profile_via_ctypes("/opt/axon/libaxon_pjrt.so")
        )
    except Exception:
        pass


def _build_program(v_bias_nonzero: bool, debug: bool = False):
    from contextlib import ExitStack

    import concourse.bass as bass
    import concourse.tile as tile
    from concourse import bacc, mybir

    f32 = mybir.dt.float32
    bf16 = mybir.dt.bfloat16
    Exp = mybir.ActivationFunctionType.Exp
    add = mybir.AluOpType.add

    nc = bacc.Bacc("TRN2", target_bir_lowering=False, debug=False,
                   num_devices=N_CORES)

    xT_d = nc.dram_tensor("xT", [C, N], bf16, kind="ExternalInput").ap()
    wqk_d = nc.dram_tensor("wqk", [C, 512], bf16, kind="ExternalInput").ap()
    wv_d = nc.dram_tensor("wv", [C, 256], bf16, kind="ExternalInput").ap()
    wp_d = nc.dram_tensor("wp", [256, C], bf16, kind="ExternalInput").ap()
    bqk_d = nc.dram_tensor("bqk", [512, 1], f32, kind="ExternalInput").ap()
    bv_d = nc.dram_tensor("bv", [64, 4], f32, kind="ExternalInput").ap()
    y_d = nc.dram_tensor("y", [N, C], f32, kind="ExternalOutput").ap()
    warm_d = nc.dram_tensor("warm", [1, 8], f32, kind="ExternalOutput").ap()
    if debug:
        dbg_qkT = nc.dram_tensor("dbg_qkT", [128, 4, N], bf16,
                                 kind="ExternalOutput").ap()
        dbg_v = nc.dram_tensor("dbg_v", [128, MT, HL * 65], bf16,
                               kind="ExternalOutput").ap()
        dbg_out = nc.dram_tensor("dbg_out", [128, 2, N], bf16,
                                 kind="ExternalOutput").ap()
        dbg_zr = nc.dram_tensor("dbg_zr", [16, 512], f32,
                                kind="ExternalOutput").ap()
        dbg_zb = nc.dram_tensor("dbg_zb", [16, 512], f32,
                                kind="ExternalOutput").ap()

    with tile.TileContext(nc) as tc, ExitStack() as ctx:
        persist = ctx.enter_context(tc.tile_pool(name="persist", bufs=1))
        # PSUM budget (8 banks): s 2x3 + pv 2x1. The s slots are shared by
        # scores / qkv / v / proj (all released by fast ACT/DVE drains, no
        # dependency cycles); pv holds only the long-lived PV accumulators.
        pv_pool = ctx.enter_context(
            tc.tile_pool(name="pv", bufs=2, space="PSUM"))
        s_pool = ctx.enter_context(
            tc.tile_pool(name="s", bufs=2, space="PSUM"))
        mm_pool = s_pool
        es_pool = ctx.enter_context(tc.tile_pool(name="es", bufs=22))
        z_pool = ctx.enter_context(tc.tile_pool(name="z", bufs=3))
        y_pool = ctx.enter_context(tc.tile_pool(name="ysb", bufs=3))
        zd_pool = ctx.enter_context(
            tc.tile_pool(name="zd", bufs=4, space="DRAM"))

        xT = persist.tile([128, KT, N], bf16)
        wqk = persist.tile([128, KT, 512], bf16)
        wv = persist.tile([128, KT, 256], bf16)
        wp = persist.tile([128, 2, C], bf16)
        bq = persist.tile([128, 4], f32)
        bv = persist.tile([64, 4], f32) if v_bias_nonzero else None
        # q/k activations split into per-(dim-tile, token-chunk) tiles so the
        # scheduler releases attention matmuls as soon as each chunk lands
        qkT = [[persist.tile([128, 512], bf16, name=f"qkT{nt}_{mc}")
                for mc in range(4)] for nt in range(4)]
        v_sb = persist.tile([128, MT, HL * 65], bf16)
        out_sb = persist.tile([128, 2, N], bf16)
        warm_sb = persist.tile([1, 8], f32)

        # few big input DMAs (per-tile trickle suffers multi-us queue
        # bubbles between transfers); xT split across two queues
        xT_r = xT_d.rearrange("(t p) n -> p t n", p=128)
        nc.gpsimd.dma_start(wqk[:], wqk_d.rearrange("(t p) n -> p t n", p=128))
        nc.sync.dma_start(xT[:, 0:4, :], xT_r[:, 0:4, :])
        nc.scalar.dma_start(xT[:, 4:8, :], xT_r[:, 4:8, :])
        nc.gpsimd.dma_start(wv[:], wv_d.rearrange("(t p) n -> p t n", p=128))
        nc.gpsimd.dma_start(wp[:], wp_d.rearrange("(t p) n -> p t n", p=128))
        # bqk[512,1] -> [128 partitions, 4 tiles]
        nc.sync.dma_start(bq[:], bqk_d.rearrange("(t p) o -> p (t o)", p=128))
        if v_bias_nonzero:
            # bv[64, 4]: column h = bias of head h, partitions 0-63
            nc.sync.dma_start(bv[:], bv_d[:])

        # warm-up exp (after the scalar-queue weight DMAs): pulls the ACT
        # table load off the critical path
        nc.vector.memset(warm_sb[:], 0.0)
        nc.scalar.activation(warm_sb[:], warm_sb[:], Exp)
        nc.sync.dma_start(warm_d[:], warm_sb[:])

        def qk_block(nt, mcs=range(4)):
            for mc in mcs:
                ps = mm_pool.tile([128, 512], f32, tag="s",
                                  name=f"qk{nt}_{mc}")
                for kt in range(KT):
                    nc.tensor.matmul(
                        ps[:],
                        lhsT=wqk[:, kt, nt * 128:(nt + 1) * 128],
                        rhs=xT[:, kt, mc * 512:(mc + 1) * 512],
                        start=(kt == 0), stop=(kt == KT - 1))
                nc.vector.tensor_scalar(
                    out=qkT[nt][mc][:], in0=ps[:],
                    scalar1=bq[:, nt:nt + 1], scalar2=None, op0=add)

        def v_block(mts):
            for mt in mts:
                ps = mm_pool.tile([128, 256], f32, tag="s", name=f"v{mt}")
                for kt in range(KT):
                    nc.tensor.matmul(
                        ps[:],
                        lhsT=xT[:, kt, mt * 128:(mt + 1) * 128],
                        rhs=wv[:, kt, :],
                        start=(kt == 0), stop=(kt == KT - 1))
                # v_aug per head = [v | ones]: the PV matmul then puts v at
                # psum partitions 0..63 and the denominator Z at partition 64
                dst = v_sb[:, mt, :].rearrange("p (h c) -> p h c", c=65)
                nc.vector.tensor_copy(
                    dst[:, :, 0:64], ps[:].rearrange("p (h c) -> p h c",
                                                     c=64))
                nc.vector.memset(dst[:, :, 64:65], 1.0)

        # j-tile grouping per (hp, ic) round: 3+3+3+3+2+2 = 16, sized to
        # the 3-bank psum slots -- bigger exp instructions amortize the
        # ~360ns/instruction ACT overhead
        GROUPS = [(0, 3), (3, 3), (6, 3), (9, 3), (12, 2), (14, 2)]
        NG = len(GROUPS)

        def s_group(step):
            rnd, g = step // NG, step % NG
            hp, ic = rnd // 4, rnd % 4
            j0, nj = GROUPS[g]
            ss = [s_pool.tile([128, nj * 512], f32, tag="s",
                              name=f"s{hp}_{ic}_{g}_{i}")
                  for i in range(2)]
            for jj in range(nj):
                jt = j0 + jj
                for hh in range(2):
                    po = hh * 64
                    nc.tensor.matmul(
                        ss[hh][:, jj * 512:(jj + 1) * 512],
                        lhsT=qkT[2 + hp][jt // 4][
                            po:po + 64, (jt % 4) * 128:(jt % 4 + 1) * 128],
                        rhs=qkT[hp][ic][po:po + 64, :],
                        start=True, stop=True)
            return ss

        def pv_normalize(hp, ic, pvs):
            for hh in range(2):
                # release the pv psum slot quickly with a single copy, then
                # run the whole normalize chain from SBUF off-critical-path.
                # DVE ops stay lane-aligned; cross-partition moves use DMA.
                pv = pvs[hh]
                oa = z_pool.tile([128, 512], f32, tag="oa")
                nc.vector.tensor_copy(oa[:], pv[:])
                zd = zd_pool.tile([1, 512], f32, tag="zd")
                nc.gpsimd.dma_start(zd[:], oa[64:65, :])
                zbz = z_pool.tile([64, 512], f32, tag="zbz")
                nc.gpsimd.dma_start(
                    zbz[:], zd[0:1, :].to_broadcast([64, 512]))
                zb = z_pool.tile([64, 512], f32, tag="zb")
                nc.vector.reciprocal_approx_fast(zb[:], zbz[:])
                if debug:
                    di = (hp * 4 + ic) * 2 + hh
                    nc.sync.dma_start(dbg_zr[di:di + 1, :], zb[0:1, :])
                    nc.sync.dma_start(dbg_zb[di:di + 1, :], zb[1:2, :])
                if hh == 0:
                    dst = out_sb[0:64, hp, ic * 512:(ic + 1) * 512]
                else:
                    dst = z_pool.tile([64, 512], bf16, tag="o1")
                nc.vector.tensor_mul(dst, oa[0:64, :], zb[:])
                if v_bias_nonzero:
                    h = 2 * hp + hh
                    nc.vector.tensor_scalar(
                        out=dst, in0=dst, scalar1=bv[0:64, h:h + 1],
                        scalar2=None, op0=add)
                if hh == 1:
                    nc.gpsimd.dma_start(
                        out_sb[64:128, hp, ic * 512:(ic + 1) * 512],
                        dst[:])

        def proj_block(its, pool=None, tag="s", use_act=False):
            for it in its:
                for oc in range(2):
                    ps = (pool or mm_pool).tile([128, 512], f32, tag=tag,
                                                name=f"y{it}_{oc}")
                    for ct in range(2):
                        nc.tensor.matmul(
                            ps[:],
                            lhsT=out_sb[:, ct, it * 128:(it + 1) * 128],
                            rhs=wp[:, ct, oc * 512:(oc + 1) * 512],
                            start=(ct == 0), stop=(ct == 1))
                    ysb = y_pool.tile([128, 512], f32, tag="y")
                    # in the tail (after the last exp) ACT is idle: alternate
                    # the psum-drain copy across engines to release slots 2x
                    # faster; during the main phase keep ACT exp-only
                    if use_act and oc == 1:
                        nc.scalar.copy(ysb[:], ps[:])
                    else:
                        nc.vector.tensor_copy(ysb[:], ps[:])
                    nc.sync.dma_start(
                        y_d[it * 128:(it + 1) * 128,
                            oc * 512:(oc + 1) * 512],
                        ysb[:])

        # Program order must be semantic order (Tile deps are program-order
        # RAW/WAR). Attention is wrapped in high_priority so the scheduler
        # runs it as soon as its inputs land, with v / qk13 / proj filling
        # the PE gaps under the ACT exp pace.
        # interleave k/q chunk emission: S matmuls of round (0, ic) become
        # ready incrementally via subtile deps (k chunk mc covers j-tiles
        # 4mc..4mc+3; q chunk mc is exactly i-chunk ic=mc)
        for mc in range(4):
            qk_block(2, [mc])          # k for heads 0,1
            qk_block(0, [mc])          # q for heads 0,1
        # v and the second head-pair's q/k are gap-fillers: park them in a
        # low-priority band well above the attention pipeline's priorities
        # so their 8-matmul units never jam the PE queue between score
        # groups (deps still order them correctly).
        with tc.high_priority(offset=-20000):
            v_block(range(MT))
            qk_block(3)                # k for heads 2,3
            qk_block(1)                # q for heads 2,3

        # One flat software pipeline across all 8 (hp, ic) rounds: scores
        # stay two groups ahead of PV globally, so round boundaries never
        # head-of-line-block the exp stream on the in-order PE queue.
        NSTEP = 8 * NG
        with tc.high_priority():
            ss_q = {0: s_group(0), 1: s_group(1)}
            pvs = None
            for st in range(NSTEP):
                rnd, g = st // NG, st % NG
                hp, ic = rnd // 4, rnd % 4
                j0, nj = GROUPS[g]
                if g == 0:
                    pvs = [pv_pool.tile([128, 512], f32, tag="pv",
                                        name=f"pv{hp}_{ic}_{i}")
                           for i in range(2)]
                ess = []
                for hh in range(2):
                    es = es_pool.tile([128, nj * 512], bf16, tag="es")
                    nc.scalar.activation(es[:], ss_q[st % 2][hh][:], Exp,
                                         scale=SCALE)
                    ess.append(es)
                if st + 2 < NSTEP:
                    ss_q[st % 2] = s_group(st + 2)
                for jj in range(nj):
                    jt = j0 + jj
                    for hh in range(2):
                        h = 2 * hp + hh
                        nc.tensor.matmul(
                            pvs[hh][0:65, :],
                            lhsT=v_sb[:, jt, h * 65:(h + 1) * 65],
                            rhs=ess[hh][:, jj * 512:(jj + 1) * 512],
                            start=(jt == 0), stop=(jt == MT - 1))
                if g == NG - 1:
                    pv_normalize(hp, ic, pvs)
        for ic in range(3):
            proj_block(range(ic * 4, (ic + 1) * 4))
        # the last i-chunk's projection runs in the tail when the score psum
        # banks are free — use them for a wider psum rotation
        proj_block(range(12, 14))
        proj_block(range(14, 16), pool=s_pool, tag="s")

        if debug:
            for nt in range(4):
                for mc in range(4):
                    nc.sync.dma_start(
                        dbg_qkT[:, nt, mc * 512:(mc + 1) * 512],
                        qkT[nt][mc][:])
            nc.sync.dma_start(dbg_v[:], v_sb[:])
            nc.sync.dma_start(dbg_out[:], out_sb[:])

    nc.compile()
    return nc


def _prep_inputs(x, w_qkv, b_qkv, w_proj):
    """Build the 8 per-core input maps (host-side shard + transpose + cast)."""
    w3 = w_qkv.reshape(C, 3, H, D)
    b3 = b_qkv.reshape(3, H, D)
    in_maps = []
    for c in range(N_CORES):
        b, g = divmod(c, 4)
        hs = slice(g * HL, (g + 1) * HL)
        wq = w3[:, 0, hs, :].reshape(C, 256)
        wk = w3[:, 1, hs, :].reshape(C, 256)
        wv = w3[:, 2, hs, :].reshape(C, 256)
        bq = b3[0, hs, :].reshape(256)
        bk = b3[1, hs, :].reshape(256)
        bv = b3[2, hs, :].reshape(256)
        # q/k transposed layout: head pair (2j, 2j+1) shares an SBUF tile
        # with partition offsets 0/64 -> natural [256,1] order is fine:
        # tile t covers dims [t*128,(t+1)*128) = heads 2t,2t+1.
        in_maps.append({
            "xT": np.ascontiguousarray(x[b].T).astype(BF),
            "wqk": np.concatenate([wq, wk], axis=1).astype(BF),
            "wv": wv.astype(BF),
            "wp": w_proj[g * 256:(g + 1) * 256, :].astype(BF),
            "bqk": np.concatenate([bq, bk]).reshape(512, 1)
                     .astype(np.float32),
            "bv": np.ascontiguousarray(bv.reshape(4, 64).T)
                    .astype(np.float32),
        })
    return in_maps


def _get_program(v_bias_nonzero: bool):
    key = ("prog", v_bias_nonzero)
    if key not in _cache:
        _install_ntff_hook()
        _cache[key] = _build_program(v_bias_nonzero)
    return _cache[key]


def run(x, w_qkv, b_qkv, w_proj, b_proj, trace=False, trace_kwargs=None):
    from concourse import bass_utils
    bass_utils.upload_artifacts = lambda tmpdir: tmpdir  # no cloud upload

    x = np.asarray(x, dtype=np.float32)
    w_qkv = np.asarray(w_qkv, dtype=np.float32)
    b_qkv = np.asarray(b_qkv, dtype=np.float32)
    w_proj = np.asarray(w_proj, dtype=np.float32)
    b_proj = np.asarray(b_proj, dtype=np.float32)

    v_bias_nonzero = bool(np.any(b_qkv.reshape(3, H, D)[2] != 0.0))
    nc = _get_program(v_bias_nonzero)
    in_maps = _prep_inputs(x, w_qkv, b_qkv, w_proj)
    res = bass_utils.run_bass_kernel_spmd(
        nc, in_maps, list(range(N_CORES)), trace=trace,
        **(trace_kwargs or {}))

    out = np.zeros((B, N, C), dtype=np.float32)
    for b in range(B):
        acc = np.zeros((N, C), dtype=np.float32)
        for g in range(4):
            acc += res.results[b * 4 + g]["y"]
        out[b] = acc + b_proj
    return out, res


def kernel(x, w_qkv, b_qkv, w_proj, b_proj):
    out, _ = run(x, w_qkv, b_qkv, w_proj, b_proj, trace=False)
    return out


# revision 56
# speedup vs baseline: 1.0218x; 1.0218x over previous
"""Multi-head attention block (qkv proj -> softmax attention -> out proj)
for B=2, N=2048, C=1024, H=16 heads of d=64, distributed over 8 NeuronCores.

Sharding: core c = (b, g) with b = c // 4 (batch), g = c % 4 (head group of
4 heads). Each core computes q/k/v for its 4 heads, full softmax attention,
and a partial output projection (its 256 input channels of w_proj). The
host sums the 4 per-batch partials and adds b_proj.

Device layout notes (per core):
  - xT [1024, 2048] = x[b].T so the contraction dim (C) lands on SBUF
    partitions for both qkv orientations.
  - q/k are produced transposed ([head_dim, tokens]); consecutive heads sit
    at partition offsets 0 / 64 so the two K=64 score matmuls of a head
    pair occupy disjoint PE row groups and run concurrently (row tiling).
  - v is produced in [tokens, head_dim] layout with an extra all-ones
    column per head; the PV matmul then yields both the unnormalized
    attention output and the softmax denominator Z in one pass.
  - softmax has no max-subtraction: scores are ~N(0,1) (|S*scale| < ~8),
    safely inside fp32 exp range.
"""

import sys
import types

import numpy as np
import ml_dtypes

B = 2
N = 2048
C = 1024
H = 16
D = 64
HL = H // 4          # heads per core = 4
SCALE = D ** -0.5
N_CORES = 8
KT = C // 128        # 8 contraction tiles
MT = N // 128        # 16 token tiles
BF = ml_dtypes.bfloat16

_cache = {}


def _install_ntff_hook():
    """Register the axon NTFF profiling hook that this image's antenv lacks
    (profiling degrades gracefully without it; needed for exec_time_ns)."""
    try:
        import antenv.axon_hooks  # noqa: F401
        return
    except ImportError:
        pass
    try:
        import antenv
        from trn_agent_boot.trn_boot import _ntff_profile_via_ctypes
    except ImportError:
        return
    mod = types.ModuleType("antenv.axon_hooks")
    _hook = [None]
    mod.set_axon_ntff_profile_hook = lambda h: _hook.__setitem__(0, h)
    mod.get_axon_ntff_profile_hook = lambda: _hook[0]
    sys.modules["antenv.axon_hooks"] = mod
    antenv.axon_hooks = mod
    try:
        mod.set_axon_ntff_profile_hook(
            _ntff_profile_via_ctypes("/opt/axon/libaxon_pjrt.so")
        )
    except Exception:
        pass


def _build_program(v_bias_nonzero: bool, debug: bool = False):
    from contextlib import ExitStack

    import concourse.bass as bass
    import concourse.tile as tile
    from concourse import bacc, mybir

    f32 = mybir.dt.float32
    bf16 = mybir.dt.bfloat16
    Exp = mybir.ActivationFunctionType.Exp
    add = mybir.AluOpType.add

    nc = bacc.Bacc("TRN2", target_bir_lowering=False, debug=False,
                   num_devices=N_CORES)

    xT_d = nc.dram_tensor("xT", [C, N], bf16, kind="ExternalInput").ap()
    wqk_d = nc.dram_tensor("wqk", [C, 512], bf16, kind="ExternalInput").ap()
    wv_d = nc.dram_tensor("wv", [C, 256], bf16, kind="ExternalInput").ap()
    wp_d = nc.dram_tensor("wp", [256, C], bf16, kind="ExternalInput").ap()
    bqk_d = nc.dram_tensor("bqk", [512, 1], f32, kind="ExternalInput").ap()
    bv_d = nc.dram_tensor("bv", [64, 4], f32, kind="ExternalInput").ap()
    y_d = nc.dram_tensor("y", [N, C], f32, kind="ExternalOutput").ap()
    warm_d = nc.dram_tensor("warm", [1, 8], f32, kind="ExternalOutput").ap()
    if debug:
        dbg_qkT = nc.dram_tensor("dbg_qkT", [128, 4, N], bf16,
                                 kind="ExternalOutput").ap()
        dbg_v = nc.dram_tensor("dbg_v", [128, MT, HL * 65], bf16,
                               kind="ExternalOutput").ap()
        dbg_out = nc.dram_tensor("dbg_out", [128, 2, N], bf16,
                                 kind="ExternalOutput").ap()
        dbg_zr = nc.dram_tensor("dbg_zr", [16, 512], f32,
                                kind="ExternalOutput").ap()
        dbg_zb = nc.dram_tensor("dbg_zb", [16, 512], f32,
                                kind="ExternalOutput").ap()

    with tile.TileContext(nc) as tc, ExitStack() as ctx:
        persist = ctx.enter_context(tc.tile_pool(name="persist", bufs=1))
        # PSUM budget (8 banks): s 2x3 + pv 2x1. The s slots are shared by
        # scores / qkv / v / proj (all released by fast ACT/DVE drains, no
        # dependency cycles); pv holds only the long-lived PV accumulators.
        pv_pool = ctx.enter_context(
            tc.tile_pool(name="pv", bufs=2, space="PSUM"))
        s_pool = ctx.enter_context(
            tc.tile_pool(name="s", bufs=2, space="PSUM"))
        mm_pool = s_pool
        es_pool = ctx.enter_context(tc.tile_pool(name="es", bufs=22))
        z_pool = ctx.enter_context(tc.tile_pool(name="z", bufs=3))
        y_pool = ctx.enter_context(tc.tile_pool(name="ysb", bufs=3))
        zd_pool = ctx.enter_context(
            tc.tile_pool(name="zd", bufs=4, space="DRAM"))

        xT = persist.tile([128, KT, N], bf16)
        wqk = persist.tile([128, KT, 512], bf16)
        wv = persist.tile([128, KT, 256], bf16)
        wp = persist.tile([128, 2, C], bf16)
        bq = persist.tile([128, 4], f32)
        bv = persist.tile([64, 4], f32) if v_bias_nonzero else None
        # q/k activations split into per-(dim-tile, token-chunk) tiles so the
        # scheduler releases attention matmuls as soon as each chunk lands
        qkT = [[persist.tile([128, 512], bf16, name=f"qkT{nt}_{mc}")
                for mc in range(4)] for nt in range(4)]
        v_sb = persist.tile([128, MT, HL * 65], bf16)
        out_sb = persist.tile([128, 2, N], bf16)
        warm_sb = persist.tile([1, 8], f32)

        # spread the input loads over engine DMA queues so the first q/k
        # matmuls aren't serialized behind one queue
        for kt in range(KT):
            nc.sync.dma_start(xT[:, kt, :], xT_d[kt * 128:(kt + 1) * 128, :])
            nc.scalar.dma_start(wqk[:, kt, :],
                                wqk_d[kt * 128:(kt + 1) * 128, :])
        for kt in range(KT):
            nc.gpsimd.dma_start(wv[:, kt, :], wv_d[kt * 128:(kt + 1) * 128, :])
        for ct in range(2):
            nc.gpsimd.dma_start(wp[:, ct, :], wp_d[ct * 128:(ct + 1) * 128, :])
        # bqk[512,1] -> [128 partitions, 4 tiles]
        nc.sync.dma_start(bq[:], bqk_d.rearrange("(t p) o -> p (t o)", p=128))
        if v_bias_nonzero:
            # bv[64, 4]: column h = bias of head h, partitions 0-63
            nc.sync.dma_start(bv[:], bv_d[:])

        # warm-up exp (after the scalar-queue weight DMAs): pulls the ACT
        # table load off the critical path
        nc.vector.memset(warm_sb[:], 0.0)
        nc.scalar.activation(warm_sb[:], warm_sb[:], Exp)
        nc.sync.dma_start(warm_d[:], warm_sb[:])

        def qk_block(nt, mcs=range(4)):
            for mc in mcs:
                ps = mm_pool.tile([128, 512], f32, tag="s",
                                  name=f"qk{nt}_{mc}")
                for kt in range(KT):
                    nc.tensor.matmul(
                        ps[:],
                        lhsT=wqk[:, kt, nt * 128:(nt + 1) * 128],
                        rhs=xT[:, kt, mc * 512:(mc + 1) * 512],
                        start=(kt == 0), stop=(kt == KT - 1))
                nc.vector.tensor_scalar(
                    out=qkT[nt][mc][:], in0=ps[:],
                    scalar1=bq[:, nt:nt + 1], scalar2=None, op0=add)

        def v_block(mts):
            for mt in mts:
                ps = mm_pool.tile([128, 256], f32, tag="s", name=f"v{mt}")
                for kt in range(KT):
                    nc.tensor.matmul(
                        ps[:],
                        lhsT=xT[:, kt, mt * 128:(mt + 1) * 128],
                        rhs=wv[:, kt, :],
                        start=(kt == 0), stop=(kt == KT - 1))
                # v_aug per head = [v | ones]: the PV matmul then puts v at
                # psum partitions 0..63 and the denominator Z at partition 64
                dst = v_sb[:, mt, :].rearrange("p (h c) -> p h c", c=65)
                nc.vector.tensor_copy(
                    dst[:, :, 0:64], ps[:].rearrange("p (h c) -> p h c",
                                                     c=64))
                nc.vector.memset(dst[:, :, 64:65], 1.0)

        # j-tile grouping per (hp, ic) round: 3+3+3+3+2+2 = 16, sized to
        # the 3-bank psum slots -- bigger exp instructions amortize the
        # ~360ns/instruction ACT overhead
        GROUPS = [(0, 3), (3, 3), (6, 3), (9, 3), (12, 2), (14, 2)]
        NG = len(GROUPS)

        def s_group(step):
            rnd, g = step // NG, step % NG
            hp, ic = rnd // 4, rnd % 4
            j0, nj = GROUPS[g]
            ss = [s_pool.tile([128, nj * 512], f32, tag="s",
                              name=f"s{hp}_{ic}_{g}_{i}")
                  for i in range(2)]
            for jj in range(nj):
                jt = j0 + jj
                for hh in range(2):
                    po = hh * 64
                    nc.tensor.matmul(
                        ss[hh][:, jj * 512:(jj + 1) * 512],
                        lhsT=qkT[2 + hp][jt // 4][
                            po:po + 64, (jt % 4) * 128:(jt % 4 + 1) * 128],
                        rhs=qkT[hp][ic][po:po + 64, :],
                        start=True, stop=True)
            return ss

        def pv_normalize(hp, ic, pvs):
            for hh in range(2):
                # release the pv psum slot quickly with a single copy, then
                # run the whole normalize chain from SBUF off-critical-path.
                # DVE ops stay lane-aligned; cross-partition moves use DMA.
                pv = pvs[hh]
                oa = z_pool.tile([128, 512], f32, tag="oa")
                nc.vector.tensor_copy(oa[:], pv[:])
                zd = zd_pool.tile([1, 512], f32, tag="zd")
                nc.gpsimd.dma_start(zd[:], oa[64:65, :])
                zbz = z_pool.tile([64, 512], f32, tag="zbz")
                nc.gpsimd.dma_start(
                    zbz[:], zd[0:1, :].to_broadcast([64, 512]))
                zb = z_pool.tile([64, 512], f32, tag="zb")
                nc.vector.reciprocal_approx_fast(zb[:], zbz[:])
                if debug:
                    di = (hp * 4 + ic) * 2 + hh
                    nc.sync.dma_start(dbg_zr[di:di + 1, :], zb[0:1, :])
                    nc.sync.dma_start(dbg_zb[di:di + 1, :], zb[1:2, :])
                if hh == 0:
                    dst = out_sb[0:64, hp, ic * 512:(ic + 1) * 512]
                else:
                    dst = z_pool.tile([64, 512], bf16, tag="o1")
                nc.vector.tensor_mul(dst, oa[0:64, :], zb[:])
                if v_bias_nonzero:
                    h = 2 * hp + hh
                    nc.vector.tensor_scalar(
                        out=dst, in0=dst, scalar1=bv[0:64, h:h + 1],
                        scalar2=None, op0=add)
                if hh == 1:
                    nc.gpsimd.dma_start(
                        out_sb[64:128, hp, ic * 512:(ic + 1) * 512],
                        dst[:])

        def proj_block(its, pool=None, tag="s", use_act=False):
            for it in its:
                for oc in range(2):
                    ps = (pool or mm_pool).tile([128, 512], f32, tag=tag,
                                                name=f"y{it}_{oc}")
                    for ct in range(2):
                        nc.tensor.matmul(
                            ps[:],
                            lhsT=out_sb[:, ct, it * 128:(it + 1) * 128],
                            rhs=wp[:, ct, oc * 512:(oc + 1) * 512],
                            start=(ct == 0), stop=(ct == 1))
                    ysb = y_pool.tile([128, 512], f32, tag="y")
                    # in the tail (after the last exp) ACT is idle: alternate
                    # the psum-drain copy across engines to release slots 2x
                    # faster; during the main phase keep ACT exp-only
                    if use_act and oc == 1:
                        nc.scalar.copy(ysb[:], ps[:])
                    else:
                        nc.vector.tensor_copy(ysb[:], ps[:])
                    nc.sync.dma_start(
                        y_d[it * 128:(it + 1) * 128,
                            oc * 512:(oc + 1) * 512],
                        ysb[:])

        # Program order must be semantic order (Tile deps are program-order
        # RAW/WAR). Attention is wrapped in high_priority so the scheduler
        # runs it as soon as its inputs land, with v / qk13 / proj filling
        # the PE gaps under the ACT exp pace.
        # interleave k/q chunk emission: S matmuls of round (0, ic) become
        # ready incrementally via subtile deps (k chunk mc covers j-tiles
        # 4mc..4mc+3; q chunk mc is exactly i-chunk ic=mc)
        for mc in range(4):
            qk_block(2, [mc])          # k for heads 0,1
            qk_block(0, [mc])          # q for heads 0,1
        # v and the second head-pair's q/k are gap-fillers: park them in a
        # low-priority band well above the attention pipeline's priorities
        # so their 8-matmul units never jam the PE queue between score
        # groups (deps still order them correctly).
        with tc.high_priority(offset=-20000):
            v_block(range(MT))
            qk_block(3)                # k for heads 2,3
            qk_block(1)                # q for heads 2,3

        # One flat software pipeline across all 8 (hp, ic) rounds: scores
        # stay two groups ahead of PV globally, so round boundaries never
        # head-of-line-block the exp stream on the in-order PE queue.
        NSTEP = 8 * NG
        with tc.high_priority():
            ss_q = {0: s_group(0), 1: s_group(1)}
            pvs = None
            for st in range(NSTEP):
                rnd, g = st // NG, st % NG
                hp, ic = rnd // 4, rnd % 4
                j0, nj = GROUPS[g]
                if g == 0:
                    pvs = [pv_pool.tile([128, 512], f32, tag="pv",
                                        name=f"pv{hp}_{ic}_{i}")
                           for i in range(2)]
                ess = []
                for hh in range(2):
                    es = es_pool.tile([128, nj * 512], bf16, tag="es")
                    nc.scalar.activation(es[:], ss_q[st % 2][hh][:], Exp,
                                         scale=SCALE)
                    ess.append(es)
                if st + 2 < NSTEP:
                    ss_q[st % 2] = s_group(st + 2)
                for jj in range(nj):
                    jt = j0 + jj
                    for hh in range(2):
                        h = 2 * hp + hh
                        nc.tensor.matmul(
                            pvs[hh][0:65, :],
                            lhsT=v_sb[:, jt, h * 65:(h + 1) * 65],
                            rhs=ess[hh][:, jj * 512:(jj + 1) * 512],
                            start=(jt == 0), stop=(jt == MT - 1))
                if g == NG - 1:
                    pv_normalize(hp, ic, pvs)
        for ic in range(3):
            proj_block(range(ic * 4, (ic + 1) * 4))
        # the last i-chunk's projection runs in the tail when the score psum
        # banks are free — use them for a wider psum rotation
        proj_block(range(12, 14))
        proj_block(range(14, 16), pool=s_pool, tag="s")

        if debug:
            for nt in range(4):
                for mc in range(4):
                    nc.sync.dma_start(
                        dbg_qkT[:, nt, mc * 512:(mc + 1) * 512],
                        qkT[nt][mc][:])
            nc.sync.dma_start(dbg_v[:], v_sb[:])
            nc.sync.dma_start(dbg_out[:], out_sb[:])

    nc.compile()
    return nc


def _prep_inputs(x, w_qkv, b_qkv, w_proj):
    """Build the 8 per-core input maps (host-side shard + transpose + cast)."""
    w3 = w_qkv.reshape(C, 3, H, D)
    b3 = b_qkv.reshape(3, H, D)
    in_maps = []
    for c in range(N_CORES):
        b, g = divmod(c, 4)
        hs = slice(g * HL, (g + 1) * HL)
        wq = w3[:, 0, hs, :].reshape(C, 256)
        wk = w3[:, 1, hs, :].reshape(C, 256)
        wv = w3[:, 2, hs, :].reshape(C, 256)
        bq = b3[0, hs, :].reshape(256)
        bk = b3[1, hs, :].reshape(256)
        bv = b3[2, hs, :].reshape(256)
        # q/k transposed layout: head pair (2j, 2j+1) shares an SBUF tile
        # with partition offsets 0/64 -> natural [256,1] order is fine:
        # tile t covers dims [t*128,(t+1)*128) = heads 2t,2t+1.
        in_maps.append({
            "xT": np.ascontiguousarray(x[b].T).astype(BF),
            "wqk": np.concatenate([wq, wk], axis=1).astype(BF),
            "wv": wv.astype(BF),
            "wp": w_proj[g * 256:(g + 1) * 256, :].astype(BF),
            "bqk": np.concatenate([bq, bk]).reshape(512, 1)
                     .astype(np.float32),
            "bv": np.ascontiguousarray(bv.reshape(4, 64).T)
                    .astype(np.float32),
        })
    return in_maps


def _get_program(v_bias_nonzero: bool):
    key = ("prog", v_bias_nonzero)
    if key not in _cache:
        _install_ntff_hook()
        _cache[key] = _build_program(v_bias_nonzero)
    return _cache[key]


def run(x, w_qkv, b_qkv, w_proj, b_proj, trace=False, trace_kwargs=None):
    from concourse import bass_utils
    bass_utils.upload_artifacts = lambda tmpdir: tmpdir  # no cloud upload

    x = np.asarray(x, dtype=np.float32)
    w_qkv = np.asarray(w_qkv, dtype=np.float32)
    b_qkv = np.asarray(b_qkv, dtype=np.float32)
    w_proj = np.asarray(w_proj, dtype=np.float32)
    b_proj = np.asarray(b_proj, dtype=np.float32)

    v_bias_nonzero = bool(np.any(b_qkv.reshape(3, H, D)[2] != 0.0))
    nc = _get_program(v_bias_nonzero)
    in_maps = _prep_inputs(x, w_qkv, b_qkv, w_proj)
    res = bass_utils.run_bass_kernel_spmd(
        nc, in_maps, list(range(N_CORES)), trace=trace,
        **(trace_kwargs or {}))

    out = np.zeros((B, N, C), dtype=np.float32)
    for b in range(B):
        acc = np.zeros((N, C), dtype=np.float32)
        for g in range(4):
            acc += res.results[b * 4 + g]["y"]
        out[b] = acc + b_proj
    return out, res


def kernel(x, w_qkv, b_qkv, w_proj, b_proj):
    out, _ = run(x, w_qkv, b_qkv, w_proj, b_proj, trace=False)
    return out


# revision 57
# speedup vs baseline: 1.0222x; 1.0004x over previous
"""Multi-head attention block (qkv proj -> softmax attention -> out proj)
for B=2, N=2048, C=1024, H=16 heads of d=64, distributed over 8 NeuronCores.

Sharding: core c = (b, g) with b = c // 4 (batch), g = c % 4 (head group of
4 heads). Each core computes q/k/v for its 4 heads, full softmax attention,
and a partial output projection (its 256 input channels of w_proj). The
host sums the 4 per-batch partials and adds b_proj.

Device layout notes (per core):
  - xT [1024, 2048] = x[b].T so the contraction dim (C) lands on SBUF
    partitions for both qkv orientations.
  - q/k are produced transposed ([head_dim, tokens]); consecutive heads sit
    at partition offsets 0 / 64 so the two K=64 score matmuls of a head
    pair occupy disjoint PE row groups and run concurrently (row tiling).
  - v is produced in [tokens, head_dim] layout with an extra all-ones
    column per head; the PV matmul then yields both the unnormalized
    attention output and the softmax denominator Z in one pass.
  - softmax has no max-subtraction: scores are ~N(0,1) (|S*scale| < ~8),
    safely inside fp32 exp range.
"""

import sys
import types

import numpy as np
import ml_dtypes

B = 2
N = 2048
C = 1024
H = 16
D = 64
HL = H // 4          # heads per core = 4
SCALE = D ** -0.5
N_CORES = 8
KT = C // 128        # 8 contraction tiles
MT = N // 128        # 16 token tiles
BF = ml_dtypes.bfloat16

_cache = {}


def _install_ntff_hook():
    """Register the axon NTFF profiling hook that this image's antenv lacks
    (profiling degrades gracefully without it; needed for exec_time_ns)."""
    try:
        import antenv.axon_hooks  # noqa: F401
        return
    except ImportError:
        pass
    try:
        import antenv
        from trn_agent_boot.trn_boot import _ntff_profile_via_ctypes
    except ImportError:
        return
    mod = types.ModuleType("antenv.axon_hooks")
    _hook = [None]
    mod.set_axon_ntff_profile_hook = lambda h: _hook.__setitem__(0, h)
    mod.get_axon_ntff_profile_hook = lambda: _hook[0]
    sys.modules["antenv.axon_hooks"] = mod
    antenv.axon_hooks = mod
    try:
        mod.set_axon_ntff_profile_hook(
            _ntff_profile_via_ctypes("/opt/axon/libaxon_pjrt.so")
        )
    except Exception:
        pass


def _build_program(v_bias_nonzero: bool, debug: bool = False):
    from contextlib import ExitStack

    import concourse.bass as bass
    import concourse.tile as tile
    from concourse import bacc, mybir

    f32 = mybir.dt.float32
    bf16 = mybir.dt.bfloat16
    Exp = mybir.ActivationFunctionType.Exp
    add = mybir.AluOpType.add

    nc = bacc.Bacc("TRN2", target_bir_lowering=False, debug=False,
                   num_devices=N_CORES)

    xT_d = nc.dram_tensor("xT", [C, N], bf16, kind="ExternalInput").ap()
    wqk_d = nc.dram_tensor("wqk", [C, 512], bf16, kind="ExternalInput").ap()
    wv_d = nc.dram_tensor("wv", [C, 256], bf16, kind="ExternalInput").ap()
    wp_d = nc.dram_tensor("wp", [256, C], bf16, kind="ExternalInput").ap()
    bqk_d = nc.dram_tensor("bqk", [512, 1], f32, kind="ExternalInput").ap()
    bv_d = nc.dram_tensor("bv", [64, 4], f32, kind="ExternalInput").ap()
    y_d = nc.dram_tensor("y", [N, C], f32, kind="ExternalOutput").ap()
    warm_d = nc.dram_tensor("warm", [1, 8], f32, kind="ExternalOutput").ap()
    if debug:
        dbg_qkT = nc.dram_tensor("dbg_qkT", [128, 4, N], bf16,
                                 kind="ExternalOutput").ap()
        dbg_v = nc.dram_tensor("dbg_v", [128, MT, HL * 65], bf16,
                               kind="ExternalOutput").ap()
        dbg_out = nc.dram_tensor("dbg_out", [128, 2, N], bf16,
                                 kind="ExternalOutput").ap()
        dbg_zr = nc.dram_tensor("dbg_zr", [16, 512], f32,
                                kind="ExternalOutput").ap()
        dbg_zb = nc.dram_tensor("dbg_zb", [16, 512], f32,
                                kind="ExternalOutput").ap()

    with tile.TileContext(nc) as tc, ExitStack() as ctx:
        persist = ctx.enter_context(tc.tile_pool(name="persist", bufs=1))
        # PSUM budget (8 banks): s 2x3 + pv 2x1. The s slots are shared by
        # scores / qkv / v / proj (all released by fast ACT/DVE drains, no
        # dependency cycles); pv holds only the long-lived PV accumulators.
        pv_pool = ctx.enter_context(
            tc.tile_pool(name="pv", bufs=2, space="PSUM"))
        s_pool = ctx.enter_context(
            tc.tile_pool(name="s", bufs=2, space="PSUM"))
        mm_pool = s_pool
        es_pool = ctx.enter_context(tc.tile_pool(name="es", bufs=22))
        z_pool = ctx.enter_context(tc.tile_pool(name="z", bufs=3))
        y_pool = ctx.enter_context(tc.tile_pool(name="ysb", bufs=3))
        zd_pool = ctx.enter_context(
            tc.tile_pool(name="zd", bufs=4, space="DRAM"))

        xT = persist.tile([128, KT, N], bf16)
        wqk = persist.tile([128, KT, 512], bf16)
        wv = persist.tile([128, KT, 256], bf16)
        wp = persist.tile([128, 2, C], bf16)
        bq = persist.tile([128, 4], f32)
        bv = persist.tile([64, 4], f32) if v_bias_nonzero else None
        # q/k activations split into per-(dim-tile, token-chunk) tiles so the
        # scheduler releases attention matmuls as soon as each chunk lands
        qkT = [[persist.tile([128, 512], bf16, name=f"qkT{nt}_{mc}")
                for mc in range(4)] for nt in range(4)]
        v_sb = persist.tile([128, MT, HL * 65], bf16)
        out_sb = persist.tile([128, 2, N], bf16)
        warm_sb = persist.tile([1, 8], f32)

        # spread the input loads over engine DMA queues so the first q/k
        # matmuls aren't serialized behind one queue
        for kt in range(KT):
            q = nc.sync if kt % 2 == 0 else nc.scalar
            q.dma_start(xT[:, kt, :], xT_d[kt * 128:(kt + 1) * 128, :])
            nc.gpsimd.dma_start(wqk[:, kt, :],
                                wqk_d[kt * 128:(kt + 1) * 128, :])
        for kt in range(KT):
            nc.gpsimd.dma_start(wv[:, kt, :], wv_d[kt * 128:(kt + 1) * 128, :])
        for ct in range(2):
            nc.gpsimd.dma_start(wp[:, ct, :], wp_d[ct * 128:(ct + 1) * 128, :])
        # bqk[512,1] -> [128 partitions, 4 tiles]
        nc.sync.dma_start(bq[:], bqk_d.rearrange("(t p) o -> p (t o)", p=128))
        if v_bias_nonzero:
            # bv[64, 4]: column h = bias of head h, partitions 0-63
            nc.sync.dma_start(bv[:], bv_d[:])

        # warm-up exp (after the scalar-queue weight DMAs): pulls the ACT
        # table load off the critical path
        nc.vector.memset(warm_sb[:], 0.0)
        nc.scalar.activation(warm_sb[:], warm_sb[:], Exp)
        nc.sync.dma_start(warm_d[:], warm_sb[:])

        def qk_block(nt, mcs=range(4)):
            for mc in mcs:
                ps = mm_pool.tile([128, 512], f32, tag="s",
                                  name=f"qk{nt}_{mc}")
                for kt in range(KT):
                    nc.tensor.matmul(
                        ps[:],
                        lhsT=wqk[:, kt, nt * 128:(nt + 1) * 128],
                        rhs=xT[:, kt, mc * 512:(mc + 1) * 512],
                        start=(kt == 0), stop=(kt == KT - 1))
                nc.vector.tensor_scalar(
                    out=qkT[nt][mc][:], in0=ps[:],
                    scalar1=bq[:, nt:nt + 1], scalar2=None, op0=add)

        def v_block(mts):
            for mt in mts:
                ps = mm_pool.tile([128, 256], f32, tag="s", name=f"v{mt}")
                for kt in range(KT):
                    nc.tensor.matmul(
                        ps[:],
                        lhsT=xT[:, kt, mt * 128:(mt + 1) * 128],
                        rhs=wv[:, kt, :],
                        start=(kt == 0), stop=(kt == KT - 1))
                # v_aug per head = [v | ones]: the PV matmul then puts v at
                # psum partitions 0..63 and the denominator Z at partition 64
                dst = v_sb[:, mt, :].rearrange("p (h c) -> p h c", c=65)
                nc.vector.tensor_copy(
                    dst[:, :, 0:64], ps[:].rearrange("p (h c) -> p h c",
                                                     c=64))
                nc.vector.memset(dst[:, :, 64:65], 1.0)

        # j-tile grouping per (hp, ic) round: 3+3+3+3+2+2 = 16, sized to
        # the 3-bank psum slots -- bigger exp instructions amortize the
        # ~360ns/instruction ACT overhead
        GROUPS = [(0, 3), (3, 3), (6, 3), (9, 3), (12, 2), (14, 2)]
        NG = len(GROUPS)

        def s_group(step):
            rnd, g = step // NG, step % NG
            hp, ic = rnd // 4, rnd % 4
            j0, nj = GROUPS[g]
            ss = [s_pool.tile([128, nj * 512], f32, tag="s",
                              name=f"s{hp}_{ic}_{g}_{i}")
                  for i in range(2)]
            for jj in range(nj):
                jt = j0 + jj
                for hh in range(2):
                    po = hh * 64
                    nc.tensor.matmul(
                        ss[hh][:, jj * 512:(jj + 1) * 512],
                        lhsT=qkT[2 + hp][jt // 4][
                            po:po + 64, (jt % 4) * 128:(jt % 4 + 1) * 128],
                        rhs=qkT[hp][ic][po:po + 64, :],
                        start=True, stop=True)
            return ss

        def pv_normalize(hp, ic, pvs):
            for hh in range(2):
                # release the pv psum slot quickly with a single copy, then
                # run the whole normalize chain from SBUF off-critical-path.
                # DVE ops stay lane-aligned; cross-partition moves use DMA.
                pv = pvs[hh]
                oa = z_pool.tile([128, 512], f32, tag="oa")
                nc.vector.tensor_copy(oa[:], pv[:])
                zd = zd_pool.tile([1, 512], f32, tag="zd")
                nc.sync.dma_start(zd[:], oa[64:65, :])
                zbz = z_pool.tile([64, 512], f32, tag="zbz")
                nc.sync.dma_start(
                    zbz[:], zd[0:1, :].to_broadcast([64, 512]))
                zb = z_pool.tile([64, 512], f32, tag="zb")
                nc.vector.reciprocal_approx_fast(zb[:], zbz[:])
                if debug:
                    di = (hp * 4 + ic) * 2 + hh
                    nc.sync.dma_start(dbg_zr[di:di + 1, :], zb[0:1, :])
                    nc.sync.dma_start(dbg_zb[di:di + 1, :], zb[1:2, :])
                if hh == 0:
                    dst = out_sb[0:64, hp, ic * 512:(ic + 1) * 512]
                else:
                    dst = z_pool.tile([64, 512], bf16, tag="o1")
                nc.vector.tensor_mul(dst, oa[0:64, :], zb[:])
                if v_bias_nonzero:
                    h = 2 * hp + hh
                    nc.vector.tensor_scalar(
                        out=dst, in0=dst, scalar1=bv[0:64, h:h + 1],
                        scalar2=None, op0=add)
                if hh == 1:
                    nc.sync.dma_start(
                        out_sb[64:128, hp, ic * 512:(ic + 1) * 512],
                        dst[:])

        def proj_block(its, pool=None, tag="s", use_act=False):
            for it in its:
                for oc in range(2):
                    ps = (pool or mm_pool).tile([128, 512], f32, tag=tag,
                                                name=f"y{it}_{oc}")
                    for ct in range(2):
                        nc.tensor.matmul(
                            ps[:],
                            lhsT=out_sb[:, ct, it * 128:(it + 1) * 128],
                            rhs=wp[:, ct, oc * 512:(oc + 1) * 512],
                            start=(ct == 0), stop=(ct == 1))
                    ysb = y_pool.tile([128, 512], f32, tag="y")
                    # in the tail (after the last exp) ACT is idle: alternate
                    # the psum-drain copy across engines to release slots 2x
                    # faster; during the main phase keep ACT exp-only
                    if use_act and oc == 1:
                        nc.scalar.copy(ysb[:], ps[:])
                    else:
                        nc.vector.tensor_copy(ysb[:], ps[:])
                    nc.sync.dma_start(
                        y_d[it * 128:(it + 1) * 128,
                            oc * 512:(oc + 1) * 512],
                        ysb[:])

        # Program order must be semantic order (Tile deps are program-order
        # RAW/WAR). Attention is wrapped in high_priority so the scheduler
        # runs it as soon as its inputs land, with v / qk13 / proj filling
        # the PE gaps under the ACT exp pace.
        # interleave k/q chunk emission: S matmuls of round (0, ic) become
        # ready incrementally via subtile deps (k chunk mc covers j-tiles
        # 4mc..4mc+3; q chunk mc is exactly i-chunk ic=mc)
        for mc in range(4):
            qk_block(2, [mc])          # k for heads 0,1
            qk_block(0, [mc])          # q for heads 0,1
        # v and the second head-pair's q/k are gap-fillers: park them in a
        # low-priority band well above the attention pipeline's priorities
        # so their 8-matmul units never jam the PE queue between score
        # groups (deps still order them correctly).
        with tc.high_priority(offset=-20000):
            v_block(range(MT))
            qk_block(3)                # k for heads 2,3
            qk_block(1)                # q for heads 2,3

        # One flat software pipeline across all 8 (hp, ic) rounds: scores
        # stay two groups ahead of PV globally, so round boundaries never
        # head-of-line-block the exp stream on the in-order PE queue.
        NSTEP = 8 * NG
        with tc.high_priority():
            ss_q = {0: s_group(0), 1: s_group(1)}
            pvs = None
            for st in range(NSTEP):
                rnd, g = st // NG, st % NG
                hp, ic = rnd // 4, rnd % 4
                j0, nj = GROUPS[g]
                if g == 0:
                    pvs = [pv_pool.tile([128, 512], f32, tag="pv",
                                        name=f"pv{hp}_{ic}_{i}")
                           for i in range(2)]
                ess = []
                for hh in range(2):
                    es = es_pool.tile([128, nj * 512], bf16, tag="es")
                    nc.scalar.activation(es[:], ss_q[st % 2][hh][:], Exp,
                                         scale=SCALE)
                    ess.append(es)
                if st + 2 < NSTEP:
                    ss_q[st % 2] = s_group(st + 2)
                for jj in range(nj):
                    jt = j0 + jj
                    for hh in range(2):
                        h = 2 * hp + hh
                        nc.tensor.matmul(
                            pvs[hh][0:65, :],
                            lhsT=v_sb[:, jt, h * 65:(h + 1) * 65],
                            rhs=ess[hh][:, jj * 512:(jj + 1) * 512],
                            start=(jt == 0), stop=(jt == MT - 1))
                if g == NG - 1:
                    pv_normalize(hp, ic, pvs)
        for ic in range(3):
            proj_block(range(ic * 4, (ic + 1) * 4))
        # the last i-chunk's projection runs in the tail when the score psum
        # banks are free — use them for a wider psum rotation
        proj_block(range(12, 14))
        proj_block(range(14, 16), pool=s_pool, tag="s")

        if debug:
            for nt in range(4):
                for mc in range(4):
                    nc.sync.dma_start(
                        dbg_qkT[:, nt, mc * 512:(mc + 1) * 512],
                        qkT[nt][mc][:])
            nc.sync.dma_start(dbg_v[:], v_sb[:])
            nc.sync.dma_start(dbg_out[:], out_sb[:])

    nc.compile()
    return nc


def _prep_inputs(x, w_qkv, b_qkv, w_proj):
    """Build the 8 per-core input maps (host-side shard + transpose + cast)."""
    w3 = w_qkv.reshape(C, 3, H, D)
    b3 = b_qkv.reshape(3, H, D)
    in_maps = []
    for c in range(N_CORES):
        b, g = divmod(c, 4)
        hs = slice(g * HL, (g + 1) * HL)
        wq = w3[:, 0, hs, :].reshape(C, 256)
        wk = w3[:, 1, hs, :].reshape(C, 256)
        wv = w3[:, 2, hs, :].reshape(C, 256)
        bq = b3[0, hs, :].reshape(256)
        bk = b3[1, hs, :].reshape(256)
        bv = b3[2, hs, :].reshape(256)
        # q/k transposed layout: head pair (2j, 2j+1) shares an SBUF tile
        # with partition offsets 0/64 -> natural [256,1] order is fine:
        # tile t covers dims [t*128,(t+1)*128) = heads 2t,2t+1.
        in_maps.append({
            "xT": np.ascontiguousarray(x[b].T).astype(BF),
            "wqk": np.concatenate([wq, wk], axis=1).astype(BF),
            "wv": wv.astype(BF),
            "wp": w_proj[g * 256:(g + 1) * 256, :].astype(BF),
            "bqk": np.concatenate([bq, bk]).reshape(512, 1)
                     .astype(np.float32),
            "bv": np.ascontiguousarray(bv.reshape(4, 64).T)
                    .astype(np.float32),
        })
    return in_maps


def _get_program(v_bias_nonzero: bool):
    key = ("prog", v_bias_nonzero)
    if key not in _cache:
        _install_ntff_hook()
        _cache[key] = _build_program(v_bias_nonzero)
    return _cache[key]


def run(x, w_qkv, b_qkv, w_proj, b_proj, trace=False, trace_kwargs=None):
    from concourse import bass_utils
    bass_utils.upload_artifacts = lambda tmpdir: tmpdir  # no cloud upload

    x = np.asarray(x, dtype=np.float32)
    w_qkv = np.asarray(w_qkv, dtype=np.float32)
    b_qkv = np.asarray(b_qkv, dtype=np.float32)
    w_proj = np.asarray(w_proj, dtype=np.float32)
    b_proj = np.asarray(b_proj, dtype=np.float32)

    v_bias_nonzero = bool(np.any(b_qkv.reshape(3, H, D)[2] != 0.0))
    nc = _get_program(v_bias_nonzero)
    in_maps = _prep_inputs(x, w_qkv, b_qkv, w_proj)
    res = bass_utils.run_bass_kernel_spmd(
        nc, in_maps, list(range(N_CORES)), trace=trace,
        **(trace_kwargs or {}))

    out = np.zeros((B, N, C), dtype=np.float32)
    for b in range(B):
        acc = np.zeros((N, C), dtype=np.float32)
        for g in range(4):
            acc += res.results[b * 4 + g]["y"]
        out[b] = acc + b_proj
    return out, res


def kernel(x, w_qkv, b_qkv, w_proj, b_proj):
    out, _ = run(x, w_qkv, b_qkv, w_proj, b_proj, trace=False)
    return out
